# revision 6
# baseline (speedup 1.0000x reference)
"""Distributed manual-attention kernel for Trainium2 (8 NeuronCores).

Problem: q,k,v (128, 8192) f32; out = softmax(q^T k, axis=kv) @ v^T -> (8192, 128).

Strategy: shard seqlen_q across the 8 cores (1024 q columns each); k/v are
replicated.  Each core runs an independent flash-attention-style kernel:

  for each q-chunk (512 q):
    for each kv tile t (128 kv):
      S^T[t]   = k_tile^T @ q_chunk          (PE, fp32r, out (kv=128, q=512) PSUM)
      E[t]     = exp(S^T[t])                 (ACT, bf16 out, batched 3 tiles/instr)
      outT    += v^T_tile^T @ E[t]           (PE, bf16, accumulate (d=128, q=512))
      acc     += E[t]                        (DVE 2/3, GPSIMD 1/3 of batches)
    denom     = colsum(acc) -> transpose -> per-q reciprocal (PE+DVE)
    out       = transpose(outT) * recip      (PE transpose + DVE scale)

v^T is produced by DMA xbar transpose (bf16) instead of PE transposes.
No max-subtraction is needed: |scores| <= ~55, exp stays well inside f32 range,
and the result is identical after normalization.

Precision: fp32r (FP22) QK^T + bf16 scores@V gives rel err ~2e-3 vs f32 ref.
"""

import numpy as np

D = 128          # head dim
SQ = 8192        # total seqlen_q
SKV = 8192       # seqlen_kv
NCORES = 8
SQS = SQ // NCORES   # 1024 q per core
QC = 512             # q chunk (matmul moving free dim)
NQC = SQS // QC      # 2 chunks
KVT = 128            # kv tile (PE contraction / partition dim)
NKV = SKV // KVT     # 64 kv tiles
BATCH = 3            # kv tiles per exp batch (3 PSUM banks)

LAST_RESULTS = None  # BassKernelResults of the most recent run (for test.py)


def _build_nc():
    import concourse.tile as tile
    from concourse import bacc, mybir
    from concourse.masks import make_identity

    f32 = mybir.dt.float32
    f32r = mybir.dt.float32r
    bf16 = mybir.dt.bfloat16

    # Bacc (vs plain Bass) runs move_matmul_waits_to_ldweights /
    # generate_event_semaphores at finalize, which split the multi-wait
    # conditions that the self-loading fp32r matmuls cannot encode.
    nc = bacc.Bacc(None, target_bir_lowering=False)
    q_ext = nc.declare_dram_parameter("q", [D, SQS], f32, isOutput=False)
    k_ext = nc.declare_dram_parameter("k", [D, SKV], f32, isOutput=False)
    v_ext = nc.declare_dram_parameter("v", [D, SKV], f32, isOutput=False)
    out_ext = nc.declare_dram_parameter("out", [SQS, D], f32, isOutput=True)

    # kv tile batches for the exp stage: 21 batches of 3 + 1 of 1
    batches = [list(range(b, min(b + BATCH, NKV))) for b in range(0, NKV, BATCH)]

    with tile.TileContext(nc) as tc:
        with (
            tc.tile_pool(name="const", bufs=1) as constp,
            tc.tile_pool(name="inputs", bufs=1) as inputs,
            tc.tile_pool(name="work", bufs=3) as workp,
            tc.tile_pool(name="accp", bufs=2) as accp,
            tc.tile_pool(name="epi", bufs=2) as epip,
            tc.tile_pool(name="qk_ps", bufs=2, space="PSUM") as qkps,
            tc.tile_pool(name="out_ps", bufs=2, space="PSUM") as outps,
        ):
            ident = constp.tile([128, 128], f32, name="ident")
            make_identity(nc, ident)

            # mm1 inputs stored as float32r (same 4-byte layout; the PE reads
            # them at FP22 precision, 1 cycle/row instead of 4).
            q_sb = inputs.tile([D, SQS], f32r, name="q_sb")
            nc.sync.dma_start(out=q_sb, in_=q_ext[:, :].bitcast(f32r))

            k_tiles = []
            for i in range(8):
                kt = inputs.tile([D, 1024], f32r, name=f"k_sb{i}", tag=f"k_sb{i}")
                nc.sync.dma_start(
                    out=kt, in_=k_ext[:, i * 1024:(i + 1) * 1024].bitcast(f32r)
                )
                k_tiles.append(kt)

            # v: load f32, cast to bf16 (DVE), then DMA xbar transpose to v^T
            vt_tiles = []
            for i in range(8):
                vf = inputs.tile(
                    [D, 1024], f32, name=f"v_sb{i}", tag=f"v_sb{i}"
                )
                nc.sync.dma_start(out=vf, in_=v_ext[:, i * 1024:(i + 1) * 1024])
                vb = inputs.tile(
                    [D, 1024], bf16, name=f"v_bf{i}", tag=f"v_bf{i}"
                )
                nc.vector.tensor_copy(vb, vf)
                for u in range(8):
                    t = 8 * i + u
                    vt_r = inputs.tile(
                        [128, 128], bf16, name=f"vt{t}", tag=f"vt{t}"
                    )
                    nc.scalar.dma_start_transpose(
                        vt_r, vb[:, u * 128:(u + 1) * 128]
                    )
                    vt_tiles.append(vt_r)

            def mm1_lhsT(t):
                kt = k_tiles[t // 8]
                off = (t % 8) * 128
                return kt[:, off:off + 128]

            for c in range(NQC):
                q_rhs = q_sb[:, c * QC:(c + 1) * QC]
                outT_ps = outps.tile([128, QC], f32, tag="outT", name=f"outT{c}")
                # separate exp accumulators for the DVE and GPSIMD shares
                accd = accp.tile([128, BATCH * QC], f32, tag="accd", name=f"accd{c}")
                accg = accp.tile([128, BATCH * QC], f32, tag="accg", name=f"accg{c}")

                first_d = True
                first_g = True
                for bi, batch in enumerate(batches):
                    w = len(batch) * QC
                    qk_ps = qkps.tile(
                        [128, BATCH * QC], f32, tag="qk", name=f"qk{c}_{bi}"
                    )
                    for j, t in enumerate(batch):
                        nc.tensor.matmul(
                            qk_ps[:, j * QC:(j + 1) * QC],
                            lhsT=mm1_lhsT(t),
                            rhs=q_rhs,
                            start=True,
                            stop=True,
                        )
                    exp3 = workp.tile(
                        [128, BATCH * QC], bf16, tag="exp3", name=f"exp{c}_{bi}"
                    )
                    nc.scalar.activation(
                        exp3[:, :w], qk_ps[:, :w],
                        func=mybir.ActivationFunctionType.Exp,
                    )
                    for j, t in enumerate(batch):
                        nc.tensor.matmul(
                            outT_ps,
                            lhsT=vt_tiles[t],
                            rhs=exp3[:, j * QC:(j + 1) * QC],
                            start=(t == 0),
                            stop=(t == NKV - 1),
                        )
                    # exp-sum accumulation: ~1/3 of batches on GPSIMD
                    on_gp = (bi % 3 == 2)
                    acc = accg if on_gp else accd
                    eng = nc.gpsimd if on_gp else nc.vector
                    if (first_g if on_gp else first_d):
                        eng.tensor_copy(acc[:, :w], exp3[:, :w])
                        if on_gp:
                            first_g = False
                        else:
                            first_d = False
                    else:
                        eng.tensor_add(acc[:, :w], acc[:, :w], exp3[:, :w])

                # ---- epilogue: denominators ----
                acc_sum = epip.tile([128, QC], f32, tag="acc_sum", name=f"accs{c}")
                nc.vector.tensor_add(acc_sum, accd[:, 0:QC], accd[:, QC:2 * QC])
                nc.vector.tensor_add(acc_sum, acc_sum, accd[:, 2 * QC:3 * QC])
                for s3 in range(3):
                    nc.vector.tensor_add(
                        acc_sum, acc_sum, accg[:, s3 * QC:(s3 + 1) * QC]
                    )
                accT_ps = qkps.tile([128, BATCH * QC], f32, tag="qk", name=f"accT{c}")
                for s in range(4):
                    nc.tensor.transpose(
                        accT_ps[:, s * 128:(s + 1) * 128],
                        acc_sum[:, s * 128:(s + 1) * 128],
                        ident,
                    )
                denom4 = epip.tile([128, 4], f32, tag="denom4", name=f"den{c}")
                nc.vector.tensor_reduce(
                    denom4,
                    accT_ps[:, 0:QC].rearrange("p (s j) -> p s j", s=4),
                    axis=mybir.AxisListType.X,
                    op=mybir.AluOpType.add,
                )
                recip4 = epip.tile([128, 4], f32, tag="recip4", name=f"rec{c}")
                nc.vector.reciprocal(recip4, denom4)

                # ---- epilogue: transpose outT -> (q, d), normalize, store ----
                outT_sb = epip.tile([128, QC], f32, tag="outT_sb", name=f"outTs{c}")
                nc.vector.tensor_copy(outT_sb, outT_ps)
                outQ_ps = qkps.tile([128, BATCH * QC], f32, tag="qk", name=f"outQ{c}")
                for s in range(4):
                    nc.tensor.transpose(
                        outQ_ps[:, s * 128:(s + 1) * 128],
                        outT_sb[:, s * 128:(s + 1) * 128],
                        ident,
                    )
                out_sb = epip.tile([128, 4, 128], f32, tag="out_sb", name=f"outs{c}")
                for s in range(4):
                    nc.vector.tensor_scalar_mul(
                        out_sb[:, s, :],
                        outQ_ps[:, s * 128:(s + 1) * 128],
                        recip4[:, s:s + 1],
                    )
                nc.sync.dma_start(
                    out=out_ext[c * QC:(c + 1) * QC, :].rearrange(
                        "(s i) j -> i s j", s=4
                    ),
                    in_=out_sb,
                )
    return nc


def kernel(q, k, v):
    global LAST_RESULTS
    from concourse.bass_utils import run_bass_kernel_spmd

    q = np.ascontiguousarray(np.asarray(q, dtype=np.float32))
    k = np.ascontiguousarray(np.asarray(k, dtype=np.float32))
    v = np.ascontiguousarray(np.asarray(v, dtype=np.float32))

    nc = _build_nc()
    nc.finalize()  # Bacc: runs the wait-splitting/reg-alloc passes
    in_maps = [
        {
            "q": np.ascontiguousarray(q[:, i * SQS:(i + 1) * SQS]),
            "k": k,
            "v": v,
        }
        for i in range(NCORES)
    ]
    res = run_bass_kernel_spmd(nc, in_maps, core_ids=list(range(NCORES)))
    LAST_RESULTS = res
    out = np.concatenate([res.results[i]["out"] for i in range(NCORES)], axis=0)
    return out.astype(np.float32)


# revision 12
# speedup vs baseline: 1.7573x; 1.7573x over previous
"""Distributed manual-attention kernel for Trainium2 (8 NeuronCores).

Problem: q,k,v (128, 8192) f32; out = softmax(q^T k, axis=kv) @ v^T -> (8192, 128).

Strategy: shard seqlen_q across the 8 cores (1024 q columns each); k/v are
replicated.  Each core runs an independent flash-attention-style kernel:

  for each q-chunk (512 q):
    for each kv tile t (128 kv):
      S^T[t]   = k_tile^T @ q_chunk          (PE, fp32r, out (kv=128, q=512) PSUM)
      E[t]     = exp(S^T[t])                 (ACT, batched 3 tiles per instr)
      outT    += v^T_tile^T @ E[t]           (PE, fp32r, accumulate (d=128, q=512))
      acc     += E[t]                        (3 parallel chains: DVE x2, GPSIMD x1)
    denom     = colsum(acc) -> transpose -> per-q reciprocal (PE+DVE)
    out       = transpose(outT) * recip      (PE transpose + DVE scale)

Stall avoidance: inputs DMA'd in interleaved (128,512) pieces so the pipeline
starts early; warm-up matmuls keep the PE HAM clock-gate at 8/8; v^T PSUM
eviction via DMA byte-copies (fp32r == f32 bits); deep exp3 buffering.

No max-subtraction is needed: |scores| <= ~55, exp stays well inside f32 range.
fp32r (FP22 in the PE, 1 cycle/row) gives rel err ~6e-4 vs the f32 reference.
"""

import numpy as np

D = 128          # head dim
SQ = 8192        # total seqlen_q
SKV = 8192       # seqlen_kv
NCORES = 8
SQS = SQ // NCORES   # 1024 q per core
QC = 512             # q chunk (matmul moving free dim)
NQC = SQS // QC      # 2 chunks
KVT = 128            # kv tile (PE contraction / partition dim)
NKV = SKV // KVT     # 64 kv tiles
BATCH = 3            # kv tiles per exp batch (3 PSUM banks)
N_WARMUP = 10        # PE warm-up matmuls (HAM ramp)

LAST_RESULTS = None  # BassKernelResults of the most recent run (for test.py)


def _build_nc():
    import concourse.tile as tile
    from concourse import bacc, mybir
    from concourse.masks import make_identity

    f32 = mybir.dt.float32
    f32r = mybir.dt.float32r
    bf16 = mybir.dt.bfloat16

    # Bacc (vs plain Bass) runs move_matmul_waits_to_ldweights /
    # generate_event_semaphores at finalize, which split the multi-wait
    # conditions that the self-loading fp32r matmuls cannot encode.
    nc = bacc.Bacc(None, target_bir_lowering=False)
    q_ext = nc.declare_dram_parameter("q", [D, SQS], f32, isOutput=False)
    k_ext = nc.declare_dram_parameter("k", [D, SKV], f32, isOutput=False)
    v_ext = nc.declare_dram_parameter("v", [D, SKV], f32, isOutput=False)
    out_ext = nc.declare_dram_parameter("out", [SQS, D], f32, isOutput=True)

    # kv tile batches for the exp stage: 21 batches of 3 + 1 of 1
    batches = [list(range(b, min(b + BATCH, NKV))) for b in range(0, NKV, BATCH)]

    with tile.TileContext(nc) as tc:
        with (
            tc.tile_pool(name="const", bufs=1) as constp,
            tc.tile_pool(name="inputs", bufs=1) as inputs,
            tc.tile_pool(name="work", bufs=5) as workp,
            tc.tile_pool(name="accp", bufs=2) as accp,
            tc.tile_pool(name="epi", bufs=2) as epip,
            tc.tile_pool(name="qk_ps", bufs=2, space="PSUM") as qkps,
            tc.tile_pool(name="out_ps", bufs=1, space="PSUM") as outps,
            tc.tile_pool(name="misc_ps", bufs=1, space="PSUM") as miscps,
        ):
            ident = constp.tile([128, 128], f32, name="ident")
            make_identity(nc, ident)

            # ---- PE warm-up: ~10 matmuls on a zeroed scratch tile get the
            # HAM activity window busy so real matmuls run at 2.4 GHz.
            scratch = constp.tile([128, 512], bf16, name="scratch")
            nc.gpsimd.memset(scratch, 0.0)
            warm_ps = miscps.tile([128, 512], f32, tag="misc", name="warm_ps")
            for _ in range(N_WARMUP):
                nc.tensor.matmul(
                    warm_ps, lhsT=scratch[:, 0:128], rhs=scratch,
                    start=True, stop=True,
                )

            # ---- inputs: interleaved (128,512) DMA pieces; q/k as fp32r.
            q_sb = inputs.tile([D, SQS], f32r, name="q_sb")
            k_tiles = [
                inputs.tile([D, 1024], f32r, name=f"k_sb{i}", tag=f"k_sb{i}")
                for i in range(8)
            ]
            v_tiles = [
                inputs.tile([D, 1024], f32, name=f"v_sb{i}", tag=f"v_sb{i}")
                for i in range(8)
            ]

            # issue order: q halves, then k/v interleaved at the rate the
            # pipeline consumes them (k tile i covers kv batches ~[8i/3..])
            nc.sync.dma_start(out=q_sb[:, 0:512], in_=q_ext[:, 0:512].bitcast(f32r))
            nc.sync.dma_start(out=q_sb[:, 512:1024],
                              in_=q_ext[:, 512:1024].bitcast(f32r))
            order = [("k", 0), ("v", 0), ("v", 1), ("k", 1), ("v", 2), ("v", 3),
                     ("k", 2), ("v", 4), ("v", 5), ("k", 3), ("v", 6), ("v", 7),
                     ("k", 4), ("k", 5), ("k", 6), ("k", 7)]
            for kind, i in order:
                for half in range(2):
                    lo, hi = i * 1024 + half * 512, i * 1024 + (half + 1) * 512
                    if kind == "k":
                        nc.sync.dma_start(
                            out=k_tiles[i][:, half * 512:(half + 1) * 512],
                            in_=k_ext[:, lo:hi].bitcast(f32r),
                        )
                    else:
                        nc.sync.dma_start(
                            out=v_tiles[i][:, half * 512:(half + 1) * 512],
                            in_=v_ext[:, lo:hi],
                        )

            # ---- v^T: 4 PE transposes per PSUM round, evicted to SBUF by a
            # plain DMA byte-copy (fp32r tag == f32 bits).
            vt_tiles = []
            for r in range(16):
                vT_ps = miscps.tile([128, 512], f32, tag="misc", name=f"vT_ps{r}")
                for u in range(4):
                    t = 4 * r + u
                    vc = v_tiles[t // 8]
                    off = (t % 8) * 128
                    nc.tensor.transpose(
                        vT_ps[:, u * 128:(u + 1) * 128], vc[:, off:off + 128], ident
                    )
                vt_r = inputs.tile([128, 512], f32r, tag=f"vt{r}", name=f"vt{r}")
                nc.vector.tensor_copy(vt_r, vT_ps)
                vt_tiles.append(vt_r)

            def mm1_lhsT(t):
                kt = k_tiles[t // 8]
                off = (t % 8) * 128
                return kt[:, off:off + 128]

            def mm2_lhsT(t):
                vt = vt_tiles[t // 4]
                off = (t % 4) * 128
                return vt[:, off:off + 128]

            for c in range(NQC):
                q_rhs = q_sb[:, c * QC:(c + 1) * QC]
                outT_ps = outps.tile([128, QC], f32, tag="outT", name=f"outT{c}")
                # 3 independent exp-sum chains (2 on DVE, 1 on GPSIMD) so no
                # single serial add chain gates the pipeline.
                accs = {
                    "A": accp.tile([128, BATCH * QC], f32, tag="accA",
                                   name=f"accA{c}"),
                    "B": accp.tile([128, BATCH * QC], f32, tag="accB",
                                   name=f"accB{c}"),
                    "G": accp.tile([128, BATCH * QC], f32, tag="accG",
                                   name=f"accG{c}"),
                }
                started = {"A": False, "B": False, "G": False}
                dve_flip = 0

                for bi, batch in enumerate(batches):
                    w = len(batch) * QC
                    qk_ps = qkps.tile(
                        [128, BATCH * QC], f32, tag="qk", name=f"qk{c}_{bi}"
                    )
                    for j, t in enumerate(batch):
                        nc.tensor.matmul(
                            qk_ps[:, j * QC:(j + 1) * QC],
                            lhsT=mm1_lhsT(t),
                            rhs=q_rhs,
                            start=True,
                            stop=True,
                        )
                    exp3 = workp.tile(
                        [128, BATCH * QC], f32r, tag="exp3", name=f"exp{c}_{bi}"
                    )
                    nc.scalar.activation(
                        exp3[:, :w], qk_ps[:, :w],
                        func=mybir.ActivationFunctionType.Exp,
                    )
                    for j, t in enumerate(batch):
                        nc.tensor.matmul(
                            outT_ps,
                            lhsT=mm2_lhsT(t),
                            rhs=exp3[:, j * QC:(j + 1) * QC],
                            start=(t == 0),
                            stop=(t == NKV - 1),
                        )
                    # exp-sum accumulation across 3 chains
                    if bi % 5 == 2:
                        key, eng = "G", nc.gpsimd
                    else:
                        key, eng = ("A", nc.vector) if dve_flip == 0 else ("B", nc.vector)
                        dve_flip ^= 1
                    acc = accs[key]
                    exp_f32 = exp3[:, :w].bitcast(f32)
                    if not started[key]:
                        eng.tensor_copy(acc[:, :w], exp_f32)
                        started[key] = True
                    else:
                        eng.tensor_add(acc[:, :w], acc[:, :w], exp_f32)

                # ---- epilogue: denominators ----
                # fold each chain 1536 -> 512 (G on gpsimd, A/B on DVE), merge
                accA, accB, accG = accs["A"], accs["B"], accs["G"]
                nc.vector.tensor_add(accA[:, 0:QC], accA[:, 0:QC], accA[:, QC:2 * QC])
                nc.vector.tensor_add(accA[:, 0:QC], accA[:, 0:QC],
                                     accA[:, 2 * QC:3 * QC])
                nc.vector.tensor_add(accB[:, 0:QC], accB[:, 0:QC], accB[:, QC:2 * QC])
                nc.vector.tensor_add(accB[:, 0:QC], accB[:, 0:QC],
                                     accB[:, 2 * QC:3 * QC])
                nc.gpsimd.tensor_add(accG[:, 0:QC], accG[:, 0:QC],
                                     accG[:, QC:2 * QC])
                nc.gpsimd.tensor_add(accG[:, 0:QC], accG[:, 0:QC],
                                     accG[:, 2 * QC:3 * QC])
                acc_sum = epip.tile([128, QC], f32, tag="acc_sum", name=f"accs{c}")
                nc.vector.tensor_add(acc_sum, accA[:, 0:QC], accB[:, 0:QC])
                nc.vector.tensor_add(acc_sum, acc_sum, accG[:, 0:QC])

                accT_ps = miscps.tile([128, QC], f32, tag="misc", name=f"accT{c}")
                for s in range(4):
                    nc.tensor.transpose(
                        accT_ps[:, s * 128:(s + 1) * 128],
                        acc_sum[:, s * 128:(s + 1) * 128],
                        ident,
                    )
                denom4 = epip.tile([128, 4], f32, tag="denom4", name=f"den{c}")
                nc.vector.tensor_reduce(
                    denom4,
                    accT_ps.rearrange("p (s j) -> p s j", s=4),
                    axis=mybir.AxisListType.X,
                    op=mybir.AluOpType.add,
                )
                recip4 = epip.tile([128, 4], f32, tag="recip4", name=f"rec{c}")
                nc.vector.reciprocal(recip4, denom4)

                # ---- epilogue: transpose outT -> (q, d), normalize, store ----
                outT_sb = epip.tile([128, QC], f32, tag="outT_sb", name=f"outTs{c}")
                nc.vector.tensor_copy(outT_sb, outT_ps)
                outQ_ps = miscps.tile([128, QC], f32, tag="misc", name=f"outQ{c}")
                for s in range(4):
                    nc.tensor.transpose(
                        outQ_ps[:, s * 128:(s + 1) * 128],
                        outT_sb[:, s * 128:(s + 1) * 128],
                        ident,
                    )
                out_sb = epip.tile([128, 4, 128], f32, tag="out_sb", name=f"outs{c}")
                for s in range(4):
                    nc.vector.tensor_scalar_mul(
                        out_sb[:, s, :],
                        outQ_ps[:, s * 128:(s + 1) * 128],
                        recip4[:, s:s + 1],
                    )
                nc.sync.dma_start(
                    out=out_ext[c * QC:(c + 1) * QC, :].rearrange(
                        "(s i) j -> i s j", s=4
                    ),
                    in_=out_sb,
                )
    return nc


def kernel(q, k, v):
    global LAST_RESULTS
    from concourse.bass_utils import run_bass_kernel_spmd

    q = np.ascontiguousarray(np.asarray(q, dtype=np.float32))
    k = np.ascontiguousarray(np.asarray(k, dtype=np.float32))
    v = np.ascontiguousarray(np.asarray(v, dtype=np.float32))

    nc = _build_nc()
    nc.finalize()  # Bacc: runs the wait-splitting/reg-alloc passes
    in_maps = [
        {
            "q": np.ascontiguousarray(q[:, i * SQS:(i + 1) * SQS]),
            "k": k,
            "v": v,
        }
        for i in range(NCORES)
    ]
    res = run_bass_kernel_spmd(nc, in_maps, core_ids=list(range(NCORES)))
    LAST_RESULTS = res
    out = np.concatenate([res.results[i]["out"] for i in range(NCORES)], axis=0)
    return out.astype(np.float32)
